# revision 13
# baseline (speedup 1.0000x reference)
"""Trainium2 Bass kernel for nn_CrossAttention_59717225284223.

Full-input contract: kernel(**inputs) takes the complete [4,256,8192] tensors,
shards across 8 NeuronCores internally (core i -> batch i//2, N-half i%2; the
x2/KV side is recomputed per batch pair so no collectives are needed), and
returns the full [4,256,8192] float32 output.

v2.1: single activation-table set (kills ACT_TABLE_LOAD thrash), immediate
threshold, LN1->MLP transposes on the DMA xbar, hoisted kvb ones columns.

v2.2: fp8 (e4m3) DoubleRow matmuls for the x2-side K/V conv and the q conv
(2 bf16 matmuls -> 1 fp8 matmul each). fp8 weights are pre-scaled by 8 to
stay in the normal range; the 8x cancels via free scale/bias knobs
downstream (activation scale, Z-denominator folding), so no extra
elementwise passes. The MLP stays bf16: fp8 there costs 2e-2 accuracy
(measured) because the x1 residual path is noise-sensitive.

v3: phases 2/3 (attention + LN1, Vector/Scalar-heavy) and phase 4 (MLP,
TensorE-heavy) fused into one software-pipelined chunk loop - chunk c's
LN1 runs while chunk c-1's MLP occupies the PE array.
"""

import os
import sys

import numpy as np

for _p in (
    "/root/.axon_site",
    "/root/.axon_site/_ro/trn_rl_repo",
    "/opt/trn_rl_repo",
):
    if os.path.isdir(_p) and _p not in sys.path:
        sys.path.append(_p)

import concourse.bass as bass  # noqa: E402
import concourse.tile as tile  # noqa: E402
from concourse import bacc, hw_specs, mybir  # noqa: E402
from concourse.bass_utils import run_bass_kernel_spmd  # noqa: E402

# All scalar activations used below (Relu/Exp/Ln/Identity/Square) live in the
# natural_log_exp_and_others table set; the default chooser greedily picks
# per-function sets and thrashes ACT_TABLE_LOAD (1.3us each) on every
# exp<->ln switch. Restrict it to the one set that has everything.
_orig_get_act_tables = hw_specs.get_activation_tables


def _single_set_act_tables(arch):
    t = _orig_get_act_tables(arch)
    return {k: (v if k == "natural_log_exp_and_others" else set())
            for k, v in t.items()}


bacc.get_activation_tables = _single_set_act_tables

F32 = mybir.dt.float32
BF16 = mybir.dt.bfloat16
FP8 = mybir.dt.float8e4
OP = mybir.AluOpType
AF = mybir.ActivationFunctionType
DR = mybir.MatmulPerfMode.DoubleRow

B, C, N = 4, 256, 8192
H, D = 4, 64
L = N // 2          # positions per core
NT = N // 128       # x2-side 128-position tiles
NCH = L // 512      # 512-position chunks per core
LN_EPS = 1e-5
BN_EPS = 1e-5
ATTN_EPS = 1e-6
WS = 8.0            # fp8 weight pre-scale
LN_WS = float(np.log(WS))

_CACHE = {}


def _build(thr_val: float):
    nc = bacc.Bacc(None, target_bir_lowering=False)

    x1 = nc.dram_tensor("x1", [C, L], BF16, kind="ExternalInput")
    x1p8 = nc.dram_tensor("x1p8", [128, 2, L], FP8, kind="ExternalInput")
    x2p8 = nc.dram_tensor("x2p8", [128, 2, N], FP8, kind="ExternalInput")
    wkv8 = nc.dram_tensor("wkv8", [128, 2, 2 * C], FP8, kind="ExternalInput")
    wq8 = nc.dram_tensor("wq8", [128, 2, C], FP8, kind="ExternalInput")
    wa = nc.dram_tensor("wa", [C, C], BF16, kind="ExternalInput")
    w1a = nc.dram_tensor("w1a", [C, 2 * C], BF16, kind="ExternalInput")
    w1b = nc.dram_tensor("w1b", [C, 2 * C], BF16, kind="ExternalInput")
    w2 = nc.dram_tensor("w2", [2 * C, C], BF16, kind="ExternalInput")
    bkr = nc.dram_tensor("bkr", [1, C], BF16, kind="ExternalInput")
    bvr = nc.dram_tensor("bvr", [1, C], F32, kind="ExternalInput")
    bqn = nc.dram_tensor("bqn", [C, 1], F32, kind="ExternalInput")
    qb8 = nc.dram_tensor("qb8", [C, 1], F32, kind="ExternalInput")
    ba = nc.dram_tensor("ba", [C, 1], F32, kind="ExternalInput")
    hbv = nc.dram_tensor("hb", [2 * C, 1], F32, kind="ExternalInput")
    g2 = nc.dram_tensor("g2", [C, 1], F32, kind="ExternalInput")
    out = nc.dram_tensor("out", [C, L], F32, kind="ExternalOutput")

    x1r = x1[:, :].rearrange("(t p) n -> p t n", p=128)
    outr = out[:, :].rearrange("(t p) n -> p t n", p=128)

    with tile.TileContext(nc) as tc:
        with tc.tile_pool(name="consts", bufs=1) as consts, \
             tc.tile_pool(name="resident", bufs=1) as res:
            # ---- constants ----
            wkv_sb = consts.tile([128, 2, 2 * C], FP8)
            nc.sync.dma_start(out=wkv_sb, in_=wkv8[:, :, :])
            wq_sb = consts.tile([128, 2, C], FP8)
            nc.sync.dma_start(out=wq_sb, in_=wq8[:, :, :])
            wa_sb = consts.tile([128, 2, C], BF16)
            nc.sync.dma_start(out=wa_sb, in_=wa[:, :].rearrange(
                "(t p) o -> p t o", p=128))
            w1a_sb = consts.tile([128, 2, 2 * C], BF16)
            nc.sync.dma_start(out=w1a_sb, in_=w1a[:, :].rearrange(
                "(t p) o -> p t o", p=128))
            w1b_sb = consts.tile([128, 2, 2 * C], BF16)
            nc.sync.dma_start(out=w1b_sb, in_=w1b[:, :].rearrange(
                "(t p) o -> p t o", p=128))
            w2_sb = consts.tile([128, 4, C], BF16)
            nc.sync.dma_start(out=w2_sb, in_=w2[:, :].rearrange(
                "(t p) o -> p t o", p=128))
            bkr_sb = consts.tile([1, C], BF16)
            nc.sync.dma_start(out=bkr_sb, in_=bkr[:, :])
            bvr_sb = consts.tile([1, C], F32)
            nc.sync.dma_start(out=bvr_sb, in_=bvr[:, :])

            bqn_sb = consts.tile([128, 2], F32)
            qb8_sb = consts.tile([128, 2], F32)
            ba_sb = consts.tile([128, 2], F32)
            g2_sb = consts.tile([128, 2], F32)
            for t in range(2):
                sl = slice(t * 128, (t + 1) * 128)
                nc.sync.dma_start(out=bqn_sb[:, t:t + 1], in_=bqn[sl, :])
                nc.sync.dma_start(out=qb8_sb[:, t:t + 1], in_=qb8[sl, :])
                nc.sync.dma_start(out=ba_sb[:, t:t + 1], in_=ba[sl, :])
                nc.sync.dma_start(out=g2_sb[:, t:t + 1], in_=g2[sl, :])
            hb_sb = consts.tile([128, 4], F32)
            for t in range(4):
                nc.sync.dma_start(out=hb_sb[:, t:t + 1],
                                  in_=hbv[t * 128:(t + 1) * 128, :])
            ones_r = consts.tile([1, 128], BF16)
            nc.gpsimd.memset(ones_r, 1.0)
            ones_c = consts.tile([128, 1], BF16)
            nc.gpsimd.memset(ones_c, 1.0)
            lneps = consts.tile([128, 1], F32)
            nc.vector.memset(lneps, LN_EPS)
            eps11 = consts.tile([1, 1], F32)
            nc.vector.memset(eps11, LN_EPS)
            one_b = consts.tile([128, 1], F32)
            nc.vector.memset(one_b, 1.0)
            lnws_b = consts.tile([128, 1], F32)
            nc.vector.memset(lnws_b, LN_WS)

            # ---- resident activations ----
            x1_sb = res.tile([128, 2, L], BF16)
            x1p_sb = res.tile([128, 2, L], FP8)
            nc.sync.dma_start(out=x1p_sb, in_=x1p8[:, :, :])
            q_sb = res.tile([128, 2, L], BF16)
            msgn_sb = res.tile([128, 2, L], BF16)
            kvbd = res.tile([128, 2, 260], BF16)
            esum_sb = res.tile([128, 2], F32)
            # K/V staging tiles: allocated once so the ones columns
            # (512:514) are written a single time instead of per tile.
            kvb_t = [res.tile([128, 514], BF16, name=f"kvb{i}")
                     for i in range(6)]
            for kb in kvb_t:
                nc.gpsimd.memset(kb[:, 512:514], 1.0)

            # ================= phase 1: x2 side (full N) =================
            # cp = WS*(k+1) for the K half (bias row = WS*(bk+1)) and WS*v
            # for the V half; the WS factor cancels downstream via Z.
            with tc.tile_pool(name="x2p", bufs=3) as x2p, \
                 tc.tile_pool(name="sc1", bufs=8) as sc1, \
                 tc.tile_pool(name="cps", bufs=1, space="PSUM") as cps, \
                 tc.tile_pool(name="kvps", bufs=1, space="PSUM") as kvps:
                kv_ps = [kvps.tile([128, 258], F32, name=f"kv_ps{m}",
                                   tag=f"kv{m}") for m in range(2)]
                cp_t = [cps.tile([128, 2 * C], F32, name=f"cp{i}")
                        for i in range(6)]
                for ch in range(N // 512):
                    x2t = x2p.tile([128, 2, 512], FP8)
                    nc.sync.dma_start(
                        out=x2t, in_=x2p8[:, :, ch * 512:(ch + 1) * 512])
                    for s in range(4):
                        # bias rank-1 batched: shared ones stationary
                        nc.tensor.matmul(cp_t[(ch * 4 + s) % 6][:, 0:C],
                                         ones_r, bkr_sb,
                                         start=True, stop=False)
                    for s in range(4):
                        i = ch * 4 + s
                        cp = cp_t[i % 6]
                        nc.tensor.matmul(
                            cp, x2t[:, :, s * 128:(s + 1) * 128],
                            wkv_sb, start=False, stop=True, perf_mode=DR)
                        kvb = kvb_t[i % 6]
                        # WS*(elu(k)+1) = max(cp, WS*exp(min(k,0)))
                        rn = sc1.tile([128, C], F32, name="rn", tag="rn")
                        nc.scalar.activation(out=rn, in_=cp[:, 0:C],
                                             func=AF.Relu, scale=-1.0 / WS,
                                             bias=one_b)
                        ex = sc1.tile([128, C], F32, name="ex", tag="ex")
                        nc.scalar.activation(out=ex, in_=rn, func=AF.Exp,
                                             scale=-1.0, bias=lnws_b)
                        nc.vector.scalar_tensor_tensor(
                            out=kvb[:, 0:C], in0=cp[:, 0:C], scalar=0.0,
                            in1=ex, op0=OP.add, op1=OP.max)
                        nc.vector.tensor_copy(out=kvb[:, C:2 * C],
                                              in_=cp[:, C:2 * C])
                        nc.tensor.matmul(kv_ps[0], kvb[:, 0:128],
                                         kvb[:, 256:514],
                                         start=(i == 0), stop=(i == NT - 1))
                        nc.tensor.matmul(kv_ps[1], kvb[:, 128:256],
                                         kvb[:, 256:514],
                                         start=(i == 0), stop=(i == NT - 1))

                # ---- KV fixup: V-bias rank-1 term ----
                # kv entries are WS^2-scaled, esum columns WS-scaled;
                # bvr is WS*bv so the fixup lands at WS^2 as well.
                bv_bc = sc1.tile([128, C], F32, name="bvbc", tag="bvbc")
                nc.gpsimd.partition_broadcast(bv_bc, bvr_sb)
                for t in range(2):
                    nc.vector.tensor_copy(out=esum_sb[:, t:t + 1],
                                          in_=kv_ps[t][:, 256:257])
                nc.gpsimd.memset(kvbd, 0.0)
                for t in range(2):
                    for hh in range(2):
                        h = t * 2 + hh
                        rsl = slice(hh * 64, hh * 64 + 64)
                        csl = slice(h * 64, h * 64 + 64)
                        nc.vector.scalar_tensor_tensor(
                            out=kvbd[rsl, t, csl], in0=bv_bc[rsl, csl],
                            scalar=esum_sb[rsl, t:t + 1],
                            in1=kv_ps[t][rsl, csl],
                            op0=OP.mult, op1=OP.add)
                        nc.gpsimd.tensor_copy(
                            out=kvbd[rsl, t, 256 + h:257 + h],
                            in_=esum_sb[rsl, t:t + 1])

            # ====== fused phases 2-4: q conv, msg, LN1, MLP, LN2, out ======
            # chunk c's attention/LN1 (Vector/Scalar-bound) overlaps chunk
            # c-1's MLP (TensorE-bound).
            with tc.tile_pool(name="sc2", bufs=3) as sc2, \
                 tc.tile_pool(name="sc3", bufs=5) as sc3, \
                 tc.tile_pool(name="stat", bufs=4) as stat, \
                 tc.tile_pool(name="hpool", bufs=5) as hpool, \
                 tc.tile_pool(name="sc4", bufs=4) as sc4, \
                 tc.tile_pool(name="t1p", bufs=3) as t1p, \
                 tc.tile_pool(name="outp", bufs=3) as outp, \
                 tc.tile_pool(name="qaps", bufs=1, space="PSUM") as qaps, \
                 tc.tile_pool(name="msgps", bufs=1, space="PSUM") as msgps, \
                 tc.tile_pool(name="hps", bufs=2, space="PSUM") as hps, \
                 tc.tile_pool(name="augps", bufs=1, space="PSUM") as augps, \
                 tc.tile_pool(name="o2ps", bufs=1, space="PSUM") as o2ps:

                def phase23(ch):
                    sl = slice(ch * 512, (ch + 1) * 512)
                    nc.sync.dma_start(out=x1_sb[:, :, sl], in_=x1r[:, :, sl])
                    for m in range(2):
                        qp = qaps.tile([128, 512], F32, name="qp")
                        nc.tensor.matmul(
                            qp, wq_sb[:, :, m * 128:(m + 1) * 128],
                            x1p_sb[:, :, sl], start=True, stop=True,
                            perf_mode=DR)
                        rq = sc2.tile([128, 512], F32, name="rq", tag="rq")
                        nc.scalar.activation(out=rq, in_=qp, func=AF.Relu,
                                             scale=-1.0 / WS,
                                             bias=bqn_sb[:, m:m + 1])
                        exq = sc2.tile([128, 512], F32, name="exq", tag="exq")
                        nc.scalar.activation(out=exq, in_=rq, func=AF.Exp,
                                             scale=-1.0, bias=lnws_b)
                        nc.vector.scalar_tensor_tensor(
                            out=q_sb[:, m, sl], in0=qp,
                            scalar=qb8_sb[:, m:m + 1], in1=exq,
                            op0=OP.add, op1=OP.max)
                    mv8 = stat.tile([128, 4, 2], F32, name="mv8", tag="mv8")
                    mss = []
                    for s_ in range(4):
                        l0 = ch * 512 + s_ * 128
                        lsl = slice(l0, l0 + 128)
                        mp = msgps.tile([128, 260], F32, name=f"mp{s_ % 2}",
                                        tag=f"mp{s_ % 2}")
                        nc.tensor.matmul(mp, q_sb[:, 0, lsl], kvbd[:, 0, :],
                                         start=True, stop=False)
                        nc.tensor.matmul(mp, q_sb[:, 1, lsl], kvbd[:, 1, :],
                                         start=False, stop=True)
                        # scores are WS^2-scaled: sparse = s*(s > WS^2*thr);
                        # Z' = 1/(WS^3*(sparse+eps)) cancels mp's WS^3.
                        zsc = stat.tile([128, 4], F32, name="zsc", tag="zsc")
                        nc.vector.tensor_copy(out=zsc, in_=mp[:, 256:260])
                        mk = stat.tile([128, 4], F32, name="mk", tag="mk")
                        nc.vector.scalar_tensor_tensor(
                            out=mk, in0=zsc,
                            scalar=thr_val * WS * WS, in1=zsc,
                            op0=OP.is_gt, op1=OP.mult)
                        nc.vector.tensor_scalar(
                            out=mk, in0=mk, scalar1=WS,
                            scalar2=ATTN_EPS * WS ** 3,
                            op0=OP.mult, op1=OP.add)
                        zt = stat.tile([128, 4], F32, name="zt",
                                       tag=f"zt{s_ % 2}")
                        nc.vector.reciprocal_approx_fast(out=zt, in_=mk)
                        ms = sc3.tile([128, C], F32, name="ms", tag="ms")
                        zb = bass.AP(tensor=zt.tensor, offset=zt.offset,
                                     ap=[list(zt.ap[0]), list(zt.ap[1]),
                                         [0, 64]])
                        nc.vector.tensor_tensor(
                            out=ms.rearrange("p (h d) -> p h d", h=4),
                            in0=mp[:, 0:256].rearrange("p (h d) -> p h d", h=4),
                            in1=zb, op=OP.mult)
                        st6 = stat.tile([128, 6], F32, name="st6", tag="st6")
                        nc.vector.bn_stats(out=st6, in_=ms)
                        nc.vector.bn_aggr(out=mv8[:, s_, :], in_=st6)
                        mss.append(ms)
                    # rz = 1/sqrt(var+eps) = exp(-0.5*ln(var+eps))
                    lnv = stat.tile([128, 4], F32, name="lnv", tag="lnv")
                    nc.scalar.activation(out=lnv, in_=mv8[:, :, 1],
                                         func=AF.Ln, bias=lneps)
                    rz = stat.tile([128, 4], F32, name="rz", tag="rz")
                    nc.scalar.activation(out=rz, in_=lnv, func=AF.Exp,
                                         scale=-0.5)
                    # nmz = -mean*rz, so msn = ms*rz + nmz on the Scalar LUT
                    nmz = stat.tile([128, 4], F32, name="nmz", tag="nmz")
                    nc.vector.scalar_tensor_tensor(
                        out=nmz, in0=mv8[:, :, 0], scalar=-1.0, in1=rz,
                        op0=OP.mult, op1=OP.mult)
                    for s_ in range(4):
                        l0 = ch * 512 + s_ * 128
                        lsl = slice(l0, l0 + 128)
                        msn = sc3.tile([128, C], BF16, name="msn", tag="msn")
                        nc.scalar.activation(
                            out=msn, in_=mss[s_], func=AF.Identity,
                            scale=rz[:, s_:s_ + 1], bias=nmz[:, s_:s_ + 1])
                        # LN1 -> MLP layout flip on the DMA xbar
                        for t in range(2):
                            nc.sync.dma_start_transpose(
                                out=msgn_sb[:, t, lsl],
                                in_=msn[:, t * 128:(t + 1) * 128])

                def phase4(ch):
                    sl = slice(ch * 512, (ch + 1) * 512)
                    hsb = []
                    for m in range(4):
                        mc = slice(m * 128, (m + 1) * 128)
                        hp = hps.tile([128, 512], F32, name="hp")
                        nc.tensor.matmul(hp, w1a_sb[:, 0, mc], x1_sb[:, 0, sl],
                                         start=True, stop=False)
                        nc.tensor.matmul(hp, w1a_sb[:, 1, mc], x1_sb[:, 1, sl],
                                         start=False, stop=False)
                        nc.tensor.matmul(hp, w1b_sb[:, 0, mc],
                                         msgn_sb[:, 0, sl],
                                         start=False, stop=False)
                        nc.tensor.matmul(hp, w1b_sb[:, 1, mc],
                                         msgn_sb[:, 1, sl],
                                         start=False, stop=True)
                        ht = hpool.tile([128, 512], BF16, name="ht")
                        if m % 2 == 0:
                            nc.scalar.activation(out=ht, in_=hp, func=AF.Relu,
                                                 bias=hb_sb[:, m:m + 1])
                        else:
                            nc.vector.tensor_scalar(
                                out=ht, in0=hp, scalar1=hb_sb[:, m:m + 1],
                                scalar2=0.0, op0=OP.add, op1=OP.max)
                        hsb.append(ht)
                    # W2 flipped to [pos, och] (h tiles stationary, g2-folded
                    # w2 moving): LN2 reduces along the free dim, so rstd is a
                    # per-partition scalar - no ones-matmul / broadcast.
                    t1t = t1p.tile([128, 2, 512], BF16, name="t1t")
                    for p4 in range(2):
                        v2st = stat.tile([128, 2, 2], F32, name="v2st",
                                         tag=f"v2st{p4}")
                        o2l = []
                        for j in range(2):
                            s4 = 2 * p4 + j
                            o2t = o2ps.tile([128, 256], F32, name=f"o2t{j}",
                                            tag=f"o2t{j}")
                            psl = slice(s4 * 128, (s4 + 1) * 128)
                            for k in range(4):
                                nc.tensor.matmul(o2t, hsb[k][:, psl],
                                                 w2_sb[:, k, :],
                                                 start=(k == 0), stop=(k == 3))
                            st62 = stat.tile([128, 6], F32, name="st62",
                                             tag="st62")
                            nc.vector.bn_stats(out=st62, in_=o2t)
                            nc.vector.bn_aggr(out=v2st[:, j, :], in_=st62)
                            o2l.append(o2t)
                        lnv2 = stat.tile([128, 2], F32, name="lnv2",
                                         tag=f"lnv2{p4}")
                        nc.scalar.activation(out=lnv2, in_=v2st[:, :, 1],
                                             func=AF.Ln, bias=lneps)
                        rz2 = stat.tile([128, 2], F32, name="rz2",
                                        tag=f"rz2{p4}")
                        nc.scalar.activation(out=rz2, in_=lnv2, func=AF.Exp,
                                             scale=-0.5)
                        nmz2 = stat.tile([128, 2], F32, name="nmz2",
                                         tag=f"nmz2{p4}")
                        nc.vector.scalar_tensor_tensor(
                            out=nmz2, in0=v2st[:, :, 0], scalar=-1.0, in1=rz2,
                            op0=OP.mult, op1=OP.mult)
                        for j in range(2):
                            s4 = 2 * p4 + j
                            t1n = sc4.tile([128, 256], BF16, name="t1n",
                                           tag=f"t1n{j}")
                            if j == 0:
                                nc.scalar.activation(
                                    out=t1n, in_=o2l[j], func=AF.Identity,
                                    scale=rz2[:, j:j + 1],
                                    bias=nmz2[:, j:j + 1])
                            else:
                                nc.vector.tensor_scalar(
                                    out=t1n, in0=o2l[j],
                                    scalar1=rz2[:, j:j + 1],
                                    scalar2=nmz2[:, j:j + 1],
                                    op0=OP.mult, op1=OP.add)
                            for t in range(2):
                                nc.sync.dma_start_transpose(
                                    out=t1t[:, t, s4 * 128:(s4 + 1) * 128],
                                    in_=t1n[:, t * 128:(t + 1) * 128])
                    for m2 in range(2):
                        ap_ = augps.tile([128, 512], F32, name="augp")
                        nc.tensor.matmul(ap_, wa_sb[:, 0, m2 * 128:(m2 + 1) * 128],
                                         x1_sb[:, 0, sl], start=True, stop=False)
                        nc.tensor.matmul(ap_, wa_sb[:, 1, m2 * 128:(m2 + 1) * 128],
                                         x1_sb[:, 1, sl], start=False, stop=True)
                        ot = outp.tile([128, 512], F32, name="ot")
                        nc.vector.scalar_tensor_tensor(
                            out=ot, in0=ap_, scalar=ba_sb[:, m2:m2 + 1],
                            in1=t1t[:, m2, :],
                            op0=OP.add, op1=OP.add)
                        nc.sync.dma_start(out=outr[:, m2, sl], in_=ot)

                for ch in range(NCH):
                    phase23(ch)
                    if ch > 0:
                        phase4(ch - 1)
                phase4(NCH - 1)

    nc.compile()
    return nc


def _host_prep(inputs):
    """Fold BN/LN affine params into weights; build per-core input maps."""
    import ml_dtypes
    f32 = np.float32
    bf16 = ml_dtypes.bfloat16
    fp8 = ml_dtypes.float8_e4m3
    x1 = np.asarray(inputs["x1"], f32)
    x2 = np.asarray(inputs["x2"], f32)
    Wq, bq = np.asarray(inputs["Wq"], f32), np.asarray(inputs["bq"], f32)
    Wk, bk = np.asarray(inputs["Wk"], f32), np.asarray(inputs["bk"], f32)
    Wv, bv = np.asarray(inputs["Wv"], f32), np.asarray(inputs["bv"], f32)
    W1, W2 = np.asarray(inputs["W1"], f32), np.asarray(inputs["W2"], f32)
    g1, b1 = np.asarray(inputs["g1"], f32), np.asarray(inputs["b1"], f32)
    g2, b2 = np.asarray(inputs["g2"], f32), np.asarray(inputs["b2"], f32)
    Wa, ba = np.asarray(inputs["Wa"], f32), np.asarray(inputs["ba"], f32)
    bn_g, bn_b = np.asarray(inputs["bn_g"], f32), np.asarray(inputs["bn_b"], f32)
    bn_m, bn_v = np.asarray(inputs["bn_m"], f32), np.asarray(inputs["bn_v"], f32)

    c = lambda a: np.ascontiguousarray(a, dtype=f32)
    cb = lambda a: np.ascontiguousarray(np.asarray(a, f32).astype(bf16))
    c8 = lambda a: np.ascontiguousarray(np.asarray(a, f32).astype(fp8))

    # fp8 pair-packed weights, pre-scaled by WS ([128 pairs, 2, out])
    wkv8 = c8((np.float32(WS) * np.concatenate(
        [Wk.T, Wv.T], axis=1)).reshape(128, 2, 2 * C))
    wq8 = c8((np.float32(WS) * Wq.T).reshape(128, 2, C))

    scale_bn = bn_g / np.sqrt(bn_v + BN_EPS)
    # fold BN affine AND the +x1 residual into the aug conv
    wa_f = cb((scale_bn[:, None] * Wa + np.eye(C, dtype=f32)).T)
    ba_f = (scale_bn * ba + (bn_b - bn_m * scale_bn) + b2)[:, None]
    W1a, W1b = W1[:, :C], W1[:, C:]
    w1a = cb(W1a.T)                                          # [C, 2C]
    w1b = cb((W1b * g1[None, :]).T)                          # [C, 2C]
    hb = c((W1b @ b1)[:, None])                              # [2C, 1]
    w2c = cb(((W2 - W2.mean(axis=0, keepdims=True))
              * g2[:, None]).T)                              # [2C, C] centered, g2-folded
    shared = {
        "wkv8": wkv8,
        "wq8": wq8,
        "bqn": c(-bq[:, None]),
        "qb8": c(np.float32(WS) * (bq[:, None] + 1.0)),
        "wa": wa_f, "ba": c(ba_f),
        "w1a": w1a, "w1b": w1b, "hb": hb,
        "w2": w2c,
        "g2": c(g2[:, None]),
        "bkr": cb(np.float32(WS) * (bk[None, :] + 1.0)),
        "bvr": c(np.float32(WS) * bv[None, :]),
    }
    in_maps = []
    for core in range(8):
        b_, half = core // 2, core % 2
        m = dict(shared)
        x1c = x1[b_][:, half * L:(half + 1) * L]
        m["x1"] = np.ascontiguousarray(x1c.astype(bf16))
        m["x1p8"] = c8(x1c.reshape(128, 2, L))
        m["x2p8"] = c8(x2[b_].reshape(128, 2, N))
        in_maps.append(m)
    return in_maps


def _get_nc(thr_val: float):
    key = ("nc", thr_val)
    if key not in _CACHE:
        _CACHE[key] = _build(thr_val)
    return _CACHE[key]


def kernel(**inputs) -> np.ndarray:
    thr_val = float(np.asarray(inputs["threshold"], np.float32).reshape(-1)[0])
    nc = _get_nc(thr_val)
    in_maps = _host_prep(inputs)
    res = run_bass_kernel_spmd(nc, in_maps, core_ids=list(range(8)),
                               trace=bool(int(os.environ.get("KBENCH_TRACE", "0"))))
    if os.environ.get("KBENCH_TIME_OUT"):
        with open(os.environ["KBENCH_TIME_OUT"], "w") as f:
            f.write(str(res.exec_time_ns))
    out = np.empty((B, C, N), np.float32)
    for core in range(8):
        b_, half = core // 2, core % 2
        out[b_][:, half * L:(half + 1) * L] = res.results[core]["out"]
    return out
